# revision 19
# baseline (speedup 1.0000x reference)
"""Multi-head causal attention (B=4, T=2048, C=1024, H=16, HD=64) on 8 TRN2 NeuronCores.

Sharding: core c handles batch b = c//2 and heads hg*8..hg*8+8 where hg = c%2
(data parallel on B, tensor parallel on heads). Each core computes
qkv projection for its head group, causal attention for its 8 heads, and a
partial output projection over its 512 local channels. Host sums the two
partial projections per batch and adds the bias.

Per-core device layouts (all chosen so no on-chip transposes of x/W are needed):
  xT    [C=1024, T=2048] bf16   (x[b].T, host-transposed)
  wqkT  [C=1024, 1024]   bf16   (rows: Q of 8 heads then K of 8 heads, transposed;
                                 Q part pre-scaled by HD^-0.5)
  wvT   [C=1024, 512]    bf16
  wpT   [512, 1024]      bf16   (W_proj columns for local channels, transposed)
Stages:
  1. qkT[o, t] = W_sel @ x.T  (o: 512 Q + 512 K, head-pair h'=2p at partitions
     0-63 / 64-127 of chunk p)  and V [t, (h, d+ones)] natural with a ones
     column appended per head.
  2. Attention computes S^T directly: for each (pair, i-super of 512, j-tile
     of 128): S^T[j, i] = K_tile^T Q (row-tiled K=64 matmul pair), causal mask
     add on the diagonal block, exp on ACT (no accumulation needed) giving
     P^T[j, i] in SBUF bf16. AV accumulates O[i, d] (+ row-sum denominator via
     the ones column) with cheap N=65 matmuls: O_psum[i-tile] += P^T_tile^T
     @ [V|1]. After the j-loop, rows are normalized by the reciprocal of the
     denominator on DVE and O is DMA-transposed ([128 t, 2x64 hd] blocks) into
     oT[d, t] for the projection.
  3. y[t, :] = O @ WpT (bf16) accumulated in fp32 PSUM.
"""

import numpy as np
import ml_dtypes

B, T, C = 4, 2048, 1024
H = 16
HD = 64
NCORES = 8
P = 128

_CACHE = {}


def _build_program():
    import concourse.bass as bass
    import concourse.mybir as mybir
    import concourse.tile as tile
    from concourse import bacc
    from contextlib import ExitStack

    DT_BF = mybir.dt.bfloat16
    DT_F32 = mybir.dt.float32
    Exp = mybir.ActivationFunctionType.Exp

    nc = bacc.Bacc("TRN2", target_bir_lowering=False, num_devices=NCORES)
    xT = nc.dram_tensor("xT", [C, T], DT_BF, kind="ExternalInput")
    wqkT = nc.dram_tensor("wqkT", [C, 1024], DT_BF, kind="ExternalInput")
    wvT = nc.dram_tensor("wvT", [C, 512], DT_BF, kind="ExternalInput")
    wpT = nc.dram_tensor("wpT", [512, 1024], DT_BF, kind="ExternalInput")
    y = nc.dram_tensor("y", [T, C], DT_F32, kind="ExternalOutput")

    NT = T // P            # 16 t-tiles
    NSUP = 4               # i-supers of 512
    NPAIR = 4              # head pairs per core
    LAG = 2                # AV trails exp by this many j-tiles

    with tile.TileContext(nc) as tc, ExitStack() as ctx:
        pers = ctx.enter_context(tc.tile_pool(name="pers", bufs=1))
        xtp = ctx.enter_context(tc.tile_pool(name="xtp", bufs=2))
        ptp = ctx.enter_context(tc.tile_pool(name="ptp", bufs=2))
        obp = ctx.enter_context(tc.tile_pool(name="obp", bufs=2))
        worky = ctx.enter_context(tc.tile_pool(name="worky", bufs=2))
        small = ctx.enter_context(tc.tile_pool(name="small", bufs=8))
        sps = ctx.enter_context(tc.tile_pool(name="sps", bufs=2, space="PSUM"))
        avp = ctx.enter_context(tc.tile_pool(name="av", bufs=2, space="PSUM"))
        accp = ctx.enter_context(tc.tile_pool(name="acc", bufs=2, space="PSUM"))

        # ---- loads ----
        xt0 = xtp.tile([P, 4, T], DT_BF, tag="xtp")
        xt1 = xtp.tile([P, 4, T], DT_BF, tag="xtp")
        xtre = xT.rearrange("(o p) t -> p o t", p=P)

        def xt(cc):
            return (xt0 if cc < 4 else xt1)[:, cc % 4, :]
        wqk = pers.tile([P, 8, 1024], DT_BF, tag="wqk")
        wv = pers.tile([P, 8, 512], DT_BF, tag="wv")
        wqkre = wqkT.rearrange("(o p) f -> p o f", p=P)
        wvre = wvT.rearrange("(o p) f -> p o f", p=P)
        TH = T // 2
        for i in range(8):
            nc.sync.dma_start(wqk[:, i, :], wqkre[:, i, :])
            nc.sync.dma_start(xt(i)[:, :TH], xtre[:, i, :TH])
        for i in range(8):
            nc.sync.dma_start(wv[:, i, :], wvre[:, i, :])
            nc.sync.dma_start(xt(i)[:, TH:], xtre[:, i, TH:])
        wp = pers.tile([P, 4, 1024], DT_BF, tag="wp")

        # maskT2[j, hh, i] = -1e30 where j > i (S^T orientation: partition=j,
        # free=i), replicated for both heads so one DVE add masks both.
        maskT2 = pers.tile([P, 2, P], DT_F32, tag="maskT2")
        nc.gpsimd.memset(maskT2[:], 0.0)
        for hh in range(2):
            nc.gpsimd.affine_select(
                out=maskT2[:, hh, :],
                in_=maskT2[:, hh, :],
                compare_op=mybir.AluOpType.is_ge,
                fill=-1e30,
                base=0,
                # keep (i - j) >= 0, i.e. j <= i
                pattern=[[1, P]],
                channel_multiplier=-1,
            )

        # ---- stage 1: qkT [o, t] and V [t, (h, d|1)] per t-chunk ----
        qkT = pers.tile([P, 8, T], DT_BF, tag="qkT")
        v2 = pers.tile([P, NT, 8, 65], DT_BF, tag="v2")
        nc.gpsimd.memset(v2[:], 1.0)

        def qkv_groups(tc4):
            ts = slice(tc4 * 512, (tc4 + 1) * 512)

            def qk_group(oc):
                def go():
                    acc = accp.tile([P, 512], DT_F32, tag="acc")
                    for cc in range(8):
                        nc.tensor.matmul(
                            acc[:],
                            wqk[:, cc, oc * P:(oc + 1) * P],
                            xt(cc)[:, ts],
                            start=(cc == 0), stop=(cc == 7),
                        )
                    nc.vector.tensor_copy(qkT[:, oc, ts], acc[:])
                return go

            def v_group(tt):
                def go():
                    accv = accp.tile([P, 512], DT_F32, tag="acc")
                    for cc in range(8):
                        nc.tensor.matmul(
                            accv[:],
                            xt(cc)[:, tt * P:(tt + 1) * P],
                            wv[:, cc, :],
                            start=(cc == 0), stop=(cc == 7),
                        )
                    nc.vector.tensor_copy(
                        v2[:, tt, :, 0:64],
                        accv[:].rearrange("p (h d) -> p h d", d=64))
                return go

            return ([qk_group(oc) for oc in range(4, 8)]
                    + [qk_group(oc) for oc in range(0, 4)]
                    + [v_group(tt) for tt in range(tc4 * 4, tc4 * 4 + 4)])

        # oT keeps only 2 i-supers (written by chains of sup, read by the
        # projection woven into sup+1): [p, sup%2, hc, t_local]
        oT = pers.tile([P, 2, 4, 512], DT_BF, tag="oT")

        def chain(sup, pr, fillers):
            """Attention for head pair pr, query rows [sup*512, (sup+1)*512)."""
            qp = qkT[:, pr, :]        # [128, T]: head A at part 0-63, B at 64-127
            kp = qkT[:, 4 + pr, :]
            i0 = sup * 512
            njt = 4 * sup + 4
            oH = [avp.tile([P, 512], DT_F32, tag="av", name=f"oH{sup}_{pr}_{h}")
                  for h in range(2)]
            pTall = ptp.tile([P, njt, 2, 512], DT_BF, tag="ptp",
                             name=f"pTall{sup}_{pr}")
            nfil = len(fillers)
            fi = 0

            for jt in range(njt):
                # paced filler (before the gated QK so PE has queued work)
                while fi * njt < (jt + 1) * nfil:
                    fillers[fi]()
                    fi += 1
                ext_start = max(i0, jt * P)
                ext = i0 + 512 - ext_start
                sp = sps.tile([P, 2, 512], DT_F32, tag="sps", name=f"sp{jt}")
                for hh in range(2):
                    hsl = slice(hh * 64, hh * 64 + 64)
                    nc.tensor.matmul(
                        sp[:, hh, :ext],
                        kp[hsl, jt * P:(jt + 1) * P],
                        qp[hsl, ext_start:ext_start + ext],
                        start=True, stop=True,
                    )
                if jt >= 4 * sup:  # diagonal block at chunk cols 0:128
                    nc.vector.tensor_add(sp[:, :, 0:P], sp[:, :, 0:P], maskT2[:])
                nc.scalar.activation(pTall[:, jt, :, :ext], sp[:, :, :ext], Exp)
            while fi < nfil:
                fillers[fi]()
                fi += 1
            # AV: per (i-tile, head) a contiguous accumulation group over j.
            # Groups sharing a PSUM bank must not interleave (start=True marks
            # the whole 2KB zero region), so bursts run group-by-group.
            for itl in range(4):
                itg = sup * 4 + itl
                for hh in range(2):
                    for jt in range(itg + 1):
                        ext_start = max(i0, jt * P)
                        off = i0 + itl * P - ext_start
                        nc.tensor.matmul(
                            oH[hh][:, itl * P:itl * P + 65],
                            pTall[:, jt, hh, off:off + P],
                            v2[:, jt, 2 * pr + hh, :],
                            start=(jt == 0), stop=(jt == itg),
                        )

            # normalize rows by the ones-column denominator, pack for transpose
            rc = small.tile([P, 8], DT_F32, tag="rc")
            ob = obp.tile([P, 8, 64], DT_BF, tag="ob")  # [t, (itl, h), d]
            for hh in range(2):
                den = oH[hh][:].rearrange("p (i c) -> p i c", c=P)[:, :, 64]
                nc.vector.reciprocal_approx_fast(rc[:, hh * 4:hh * 4 + 4], den)
                for itl in range(4):
                    nc.vector.tensor_scalar_mul(
                        ob[:, itl * 2 + hh, :],
                        oH[hh][:, itl * P:itl * P + 64],
                        rc[:, hh * 4 + itl:hh * 4 + itl + 1])
            for itl in range(4):
                nc.sync.dma_start_transpose(
                    oT[:, sup % 2, pr, itl * P:(itl + 1) * P],
                    ob[:, itl * 2:itl * 2 + 2, :])

        def proj_groups(sup):
            def group(tt, oc2):
                def go():
                    ysb = worky.tile([P, 512], DT_F32, tag="ysb")
                    acc = accp.tile([P, 512], DT_F32, tag="acc")
                    for hc in range(4):
                        nc.tensor.matmul(
                            acc[:],
                            oT[:, sup % 2, hc, (tt - sup * 4) * P:
                               (tt - sup * 4 + 1) * P],
                            wp[:, hc, oc2 * 512:(oc2 + 1) * 512],
                            start=(hc == 0), stop=(hc == 3),
                        )
                    nc.vector.tensor_copy(ysb[:], acc[:])
                    nc.gpsimd.dma_start(
                        y[tt * P:(tt + 1) * P, oc2 * 512:(oc2 + 1) * 512], ysb[:])
                return go
            return [group(tt, oc2)
                    for tt in range(sup * 4, sup * 4 + 4) for oc2 in range(2)]

        wpre = wpT.rearrange("(o p) f -> p o f", p=P)
        for sup in range(NSUP):
            if sup == 0:
                g0 = qkv_groups(0)  # [k4..7, q0..3, v0..3]
                k_, q_, v_ = g0[0:4], g0[4:8], g0[8:12]
                pre = [[k_[0], q_[0]],
                       [k_[1], q_[1]],
                       [k_[2], q_[2]],
                       [k_[3], q_[3]]]
                # v0..3 must be chain(0,0)'s fillers: its AV burst reads all
                # four v2 chunks, and fillers flush before the burst.
                filler = v_ + qkv_groups(1)
            else:
                pre = [[], [], [], []]
                filler = qkv_groups(sup + 1) if sup < NSUP - 1 else []
                filler = filler + proj_groups(sup - 1)
            nfil = (len(filler) + NPAIR - 1) // NPAIR if filler else 0
            for pr in range(NPAIR):
                for g in pre[pr]:
                    g()
                chain(sup, pr, filler[pr * nfil:(pr + 1) * nfil])
            if sup == 0:
                for i in range(4):
                    nc.sync.dma_start(wp[:, i, :], wpre[:, i, :])
        for g in proj_groups(NSUP - 1):
            g()

    nc.compile()
    return nc


def _prep_inputs(x, W_qkv, W_proj):
    """Per-core host-side sharding and layout prep."""
    bf16 = ml_dtypes.bfloat16
    scale = np.float32(HD ** -0.5)
    in_maps = []
    for c in range(NCORES):
        b, hg = c // 2, c % 2
        heads = list(range(hg * 8, hg * 8 + 8))
        rq = np.concatenate([np.arange(h * 192, h * 192 + 64) for h in heads])
        rk = np.concatenate([np.arange(h * 192 + 64, h * 192 + 128) for h in heads])
        rv = np.concatenate([np.arange(h * 192 + 128, h * 192 + 192) for h in heads])
        wq = W_qkv[rq] * scale           # fold softmax scale into Q (exact: /8)
        wk = W_qkv[rk]
        wqkT = np.ascontiguousarray(np.concatenate([wq, wk], 0).T).astype(bf16)
        wvT = np.ascontiguousarray(W_qkv[rv].T).astype(bf16)
        wpT = np.ascontiguousarray(W_proj[:, hg * 512:(hg + 1) * 512].T)
        xTb = np.ascontiguousarray(x[b].T).astype(bf16)
        in_maps.append({"xT": xTb, "wqkT": wqkT, "wvT": wvT,
                        "wpT": wpT.astype(bf16)})
    return in_maps


def kernel(x, W_qkv, W_proj, b_proj):
    from concourse.bass_utils import run_bass_kernel_spmd

    x = np.asarray(x, dtype=np.float32)
    W_qkv = np.asarray(W_qkv, dtype=np.float32)
    W_proj = np.asarray(W_proj, dtype=np.float32)
    b_proj = np.asarray(b_proj, dtype=np.float32)

    if "nc" not in _CACHE:
        _CACHE["nc"] = _build_program()
    nc = _CACHE["nc"]

    in_maps = _prep_inputs(x, W_qkv, W_proj)
    res = run_bass_kernel_spmd(nc, in_maps, core_ids=list(range(NCORES)))
    out = np.empty((B, T, C), dtype=np.float32)
    for b in range(B):
        out[b] = res.results[2 * b]["y"] + res.results[2 * b + 1]["y"] + b_proj
    return out


# revision 22
# speedup vs baseline: 1.0128x; 1.0128x over previous
"""Multi-head causal attention (B=4, T=2048, C=1024, H=16, HD=64) on 8 TRN2 NeuronCores.

Sharding: core c handles batch b = c//2 and heads hg*8..hg*8+8 where hg = c%2
(data parallel on B, tensor parallel on heads). Each core computes
qkv projection for its head group, causal attention for its 8 heads, and a
partial output projection over its 512 local channels. Host sums the two
partial projections per batch and adds the bias.

Per-core device layouts (all chosen so no on-chip transposes of x/W are needed):
  xT    [C=1024, T=2048] bf16   (x[b].T, host-transposed)
  wqkT  [C=1024, 1024]   bf16   (rows: Q of 8 heads then K of 8 heads, transposed;
                                 Q part pre-scaled by HD^-0.5)
  wvT   [C=1024, 512]    bf16
  wpT   [512, 1024]      bf16   (W_proj columns for local channels, transposed)
Stages:
  1. qkT[o, t] = W_sel @ x.T  (o: 512 Q + 512 K, head-pair h'=2p at partitions
     0-63 / 64-127 of chunk p)  and V [t, (h, d+ones)] natural with a ones
     column appended per head.
  2. Attention computes S^T directly: for each (pair, i-super of 512, j-tile
     of 128): S^T[j, i] = K_tile^T Q (row-tiled K=64 matmul pair), causal mask
     add on the diagonal block, exp on ACT (no accumulation needed) giving
     P^T[j, i] in SBUF bf16. AV accumulates O[i, d] (+ row-sum denominator via
     the ones column) with cheap N=65 matmuls: O_psum[i-tile] += P^T_tile^T
     @ [V|1]. After the j-loop, rows are normalized by the reciprocal of the
     denominator on DVE and O is DMA-transposed ([128 t, 2x64 hd] blocks) into
     oT[d, t] for the projection.
  3. y[t, :] = O @ WpT (bf16) accumulated in fp32 PSUM.
"""

import numpy as np
import ml_dtypes

B, T, C = 4, 2048, 1024
H = 16
HD = 64
NCORES = 8
P = 128

_CACHE = {}


def _build_program():
    import concourse.bass as bass
    import concourse.mybir as mybir
    import concourse.tile as tile
    from concourse import bacc
    from contextlib import ExitStack

    DT_BF = mybir.dt.bfloat16
    DT_F32 = mybir.dt.float32
    Exp = mybir.ActivationFunctionType.Exp

    nc = bacc.Bacc("TRN2", target_bir_lowering=False, num_devices=NCORES)
    xT = nc.dram_tensor("xT", [C, T], DT_BF, kind="ExternalInput")
    wqkT = nc.dram_tensor("wqkT", [C, 1024], DT_BF, kind="ExternalInput")
    wvT = nc.dram_tensor("wvT", [C, 512], DT_BF, kind="ExternalInput")
    wpT = nc.dram_tensor("wpT", [512, 1024], DT_BF, kind="ExternalInput")
    y = nc.dram_tensor("y", [T, C], DT_F32, kind="ExternalOutput")

    NT = T // P            # 16 t-tiles
    NSUP = 4               # i-supers of 512
    NPAIR = 4              # head pairs per core
    LAG = 2                # AV trails exp by this many j-tiles

    with tile.TileContext(nc) as tc, ExitStack() as ctx:
        pers = ctx.enter_context(tc.tile_pool(name="pers", bufs=1))
        xtp = ctx.enter_context(tc.tile_pool(name="xtp", bufs=2))
        ptp = ctx.enter_context(tc.tile_pool(name="ptp", bufs=2))
        obp = ctx.enter_context(tc.tile_pool(name="obp", bufs=2))
        worky = ctx.enter_context(tc.tile_pool(name="worky", bufs=2))
        small = ctx.enter_context(tc.tile_pool(name="small", bufs=8))
        sps = ctx.enter_context(tc.tile_pool(name="sps", bufs=2, space="PSUM"))
        avp = ctx.enter_context(tc.tile_pool(name="av", bufs=2, space="PSUM"))
        accp = ctx.enter_context(tc.tile_pool(name="acc", bufs=2, space="PSUM"))

        # ---- loads ----
        xt0 = xtp.tile([P, 4, T], DT_BF, tag="xtp")
        xt1 = xtp.tile([P, 4, T], DT_BF, tag="xtp")
        xtre = xT.rearrange("(o p) t -> p o t", p=P)

        def xt(cc):
            return (xt0 if cc < 4 else xt1)[:, cc % 4, :]
        wqk = pers.tile([P, 8, 1024], DT_BF, tag="wqk")
        wv = pers.tile([P, 8, 512], DT_BF, tag="wv")
        wqkre = wqkT.rearrange("(o p) f -> p o f", p=P)
        wvre = wvT.rearrange("(o p) f -> p o f", p=P)
        TH = T // 2
        for i in range(8):
            nc.sync.dma_start(wqk[:, i, :], wqkre[:, i, :])
            nc.sync.dma_start(xt(i)[:, :TH], xtre[:, i, :TH])
        for i in range(8):
            nc.sync.dma_start(wv[:, i, :], wvre[:, i, :])
            nc.sync.dma_start(xt(i)[:, TH:], xtre[:, i, TH:])
        wp = pers.tile([P, 4, 1024], DT_BF, tag="wp")

        # maskT2[j, hh, i] = -1e30 where j > i (S^T orientation: partition=j,
        # free=i), replicated for both heads so one DVE add masks both.
        maskT2 = pers.tile([P, 2, P], DT_F32, tag="maskT2")
        nc.gpsimd.memset(maskT2[:], 0.0)
        for hh in range(2):
            nc.gpsimd.affine_select(
                out=maskT2[:, hh, :],
                in_=maskT2[:, hh, :],
                compare_op=mybir.AluOpType.is_ge,
                fill=-1e30,
                base=0,
                # keep (i - j) >= 0, i.e. j <= i
                pattern=[[1, P]],
                channel_multiplier=-1,
            )

        # ---- stage 1: qkT [o, t] and V [t, (h, d|1)] per t-chunk ----
        qkT = pers.tile([P, 8, T], DT_BF, tag="qkT")
        v2 = pers.tile([P, NT, 8, 65], DT_BF, tag="v2")
        nc.gpsimd.memset(v2[:], 1.0)

        def qkv_groups(tc4):
            ts = slice(tc4 * 512, (tc4 + 1) * 512)

            def qk_group(oc):
                def go():
                    acc = accp.tile([P, 512], DT_F32, tag="acc")
                    for cc in range(8):
                        nc.tensor.matmul(
                            acc[:],
                            wqk[:, cc, oc * P:(oc + 1) * P],
                            xt(cc)[:, ts],
                            start=(cc == 0), stop=(cc == 7),
                        )
                    nc.vector.tensor_copy(qkT[:, oc, ts], acc[:])
                return go

            def v_group(tt):
                def go():
                    accv = accp.tile([P, 512], DT_F32, tag="acc")
                    for cc in range(8):
                        nc.tensor.matmul(
                            accv[:],
                            xt(cc)[:, tt * P:(tt + 1) * P],
                            wv[:, cc, :],
                            start=(cc == 0), stop=(cc == 7),
                        )
                    nc.vector.tensor_copy(
                        v2[:, tt, :, 0:64],
                        accv[:].rearrange("p (h d) -> p h d", d=64))
                return go

            return ([qk_group(oc) for oc in range(4, 8)]
                    + [qk_group(oc) for oc in range(0, 4)]
                    + [v_group(tt) for tt in range(tc4 * 4, tc4 * 4 + 4)])

        # oT keeps only 2 i-supers (written by chains of sup, read by the
        # projection woven into sup+1): [p, sup%2, hc, t_local]
        oT = pers.tile([P, 2, 4, 512], DT_BF, tag="oT")

        def chain(sup, pr, fillers):
            """Attention j-loop for head pair pr, query rows [sup*512, ...).
            Returns a `finish` closure (AV burst + normalize + transpose) that
            the caller weaves into the NEXT chain's fillers, so the PE runs the
            next pair's QK while ACT drains this pair's last exps."""
            qp = qkT[:, pr, :]        # [128, T]: head A at part 0-63, B at 64-127
            kp = qkT[:, 4 + pr, :]
            i0 = sup * 512
            njt = 4 * sup + 4
            oH = [avp.tile([P, 512], DT_F32, tag="av", name=f"oH{sup}_{pr}_{h}")
                  for h in range(2)]
            pTall = ptp.tile([P, njt, 2, 512], DT_BF, tag="ptp",
                             name=f"pTall{sup}_{pr}")
            nfil = len(fillers)
            fi = 0

            for jt in range(njt):
                # paced filler (before the gated QK so PE has queued work)
                while fi * njt < (jt + 1) * nfil:
                    fillers[fi]()
                    fi += 1
                ext_start = max(i0, jt * P)
                ext = i0 + 512 - ext_start
                sp = sps.tile([P, 2, 512], DT_F32, tag="sps", name=f"sp{jt}")
                for hh in range(2):
                    hsl = slice(hh * 64, hh * 64 + 64)
                    nc.tensor.matmul(
                        sp[:, hh, :ext],
                        kp[hsl, jt * P:(jt + 1) * P],
                        qp[hsl, ext_start:ext_start + ext],
                        start=True, stop=True,
                    )
                if jt >= 4 * sup:  # diagonal block at chunk cols 0:128
                    nc.vector.tensor_add(sp[:, :, 0:P], sp[:, :, 0:P], maskT2[:])
                nc.scalar.activation(pTall[:, jt, :, :ext], sp[:, :, :ext], Exp)
            while fi < nfil:
                fillers[fi]()
                fi += 1

            def finish():
                # AV: per (i-tile, head) a contiguous accumulation group over
                # j. Groups sharing a PSUM bank must not interleave
                # (start=True marks the whole 2KB zero region), so bursts run
                # group-by-group.
                for itl in range(4):
                    itg = sup * 4 + itl
                    for hh in range(2):
                        for jt in range(itg + 1):
                            ext_start = max(i0, jt * P)
                            off = i0 + itl * P - ext_start
                            nc.tensor.matmul(
                                oH[hh][:, itl * P:itl * P + 65],
                                pTall[:, jt, hh, off:off + P],
                                v2[:, jt, 2 * pr + hh, :],
                                start=(jt == 0), stop=(jt == itg),
                            )
                # normalize rows by the ones-column denominator, pack, and
                # transpose into oT for the projection
                rc = small.tile([P, 8], DT_F32, tag="rc")
                ob = obp.tile([P, 8, 64], DT_BF, tag="ob")  # [t, (itl, h), d]
                for hh in range(2):
                    den = oH[hh][:].rearrange("p (i c) -> p i c", c=P)[:, :, 64]
                    nc.vector.reciprocal_approx_fast(rc[:, hh * 4:hh * 4 + 4], den)
                    for itl in range(4):
                        nc.vector.tensor_scalar_mul(
                            ob[:, itl * 2 + hh, :],
                            oH[hh][:, itl * P:itl * P + 64],
                            rc[:, hh * 4 + itl:hh * 4 + itl + 1])
                for itl in range(4):
                    nc.sync.dma_start_transpose(
                        oT[:, sup % 2, pr, itl * P:(itl + 1) * P],
                        ob[:, itl * 2:itl * 2 + 2, :])
            return finish

        def proj_groups(sup):
            def group(tt, oc2):
                def go():
                    ysb = worky.tile([P, 512], DT_F32, tag="ysb")
                    acc = accp.tile([P, 512], DT_F32, tag="acc")
                    for hc in range(4):
                        nc.tensor.matmul(
                            acc[:],
                            oT[:, sup % 2, hc, (tt - sup * 4) * P:
                               (tt - sup * 4 + 1) * P],
                            wp[:, hc, oc2 * 512:(oc2 + 1) * 512],
                            start=(hc == 0), stop=(hc == 3),
                        )
                    nc.vector.tensor_copy(ysb[:], acc[:])
                    nc.gpsimd.dma_start(
                        y[tt * P:(tt + 1) * P, oc2 * 512:(oc2 + 1) * 512], ysb[:])
                return go
            return [group(tt, oc2)
                    for tt in range(sup * 4, sup * 4 + 4) for oc2 in range(2)]

        wpre = wpT.rearrange("(o p) f -> p o f", p=P)
        pending = None
        for sup in range(NSUP):
            if sup == 0:
                g0 = qkv_groups(0)  # [k4..7, q0..3, v0..3]
                k_, q_, v_ = g0[0:4], g0[4:8], g0[8:12]
                pre = [[k_[0], q_[0]],
                       [k_[1], q_[1]],
                       [k_[2], q_[2]],
                       [k_[3], q_[3]]]
                # v0..3 must be chain(0,0)'s fillers: its AV burst reads all
                # four v2 chunks, and fillers flush before the burst.
                filler = v_ + qkv_groups(1)
            else:
                pre = [[], [], [], []]
                filler = qkv_groups(sup + 1) if sup < NSUP - 1 else []
                filler = filler + proj_groups(sup - 1)
            nfil = (len(filler) + NPAIR - 1) // NPAIR if filler else 0
            for pr in range(NPAIR):
                for g in pre[pr]:
                    g()
                sl = filler[pr * nfil:(pr + 1) * nfil]
                if pending is not None:
                    # weave the previous chain's finish into this chain's
                    # fillers; at (sup3, pr0) it must precede the proj(2)
                    # units whose oT inputs it writes
                    idx = min(1 if (sup < 3 or pr > 0) else 0, len(sl))
                    sl = sl[:idx] + [pending] + sl[idx:]
                pending = chain(sup, pr, sl)
            if sup == 0:
                for i in range(4):
                    nc.sync.dma_start(wp[:, i, :], wpre[:, i, :])
        pending()
        for g in proj_groups(NSUP - 1):
            g()

    nc.compile()
    return nc


def _prep_inputs(x, W_qkv, W_proj):
    """Per-core host-side sharding and layout prep."""
    bf16 = ml_dtypes.bfloat16
    scale = np.float32(HD ** -0.5)
    in_maps = []
    for c in range(NCORES):
        b, hg = c // 2, c % 2
        heads = list(range(hg * 8, hg * 8 + 8))
        rq = np.concatenate([np.arange(h * 192, h * 192 + 64) for h in heads])
        rk = np.concatenate([np.arange(h * 192 + 64, h * 192 + 128) for h in heads])
        rv = np.concatenate([np.arange(h * 192 + 128, h * 192 + 192) for h in heads])
        wq = W_qkv[rq] * scale           # fold softmax scale into Q (exact: /8)
        wk = W_qkv[rk]
        wqkT = np.ascontiguousarray(np.concatenate([wq, wk], 0).T).astype(bf16)
        wvT = np.ascontiguousarray(W_qkv[rv].T).astype(bf16)
        wpT = np.ascontiguousarray(W_proj[:, hg * 512:(hg + 1) * 512].T)
        xTb = np.ascontiguousarray(x[b].T).astype(bf16)
        in_maps.append({"xT": xTb, "wqkT": wqkT, "wvT": wvT,
                        "wpT": wpT.astype(bf16)})
    return in_maps


def kernel(x, W_qkv, W_proj, b_proj):
    from concourse.bass_utils import run_bass_kernel_spmd

    x = np.asarray(x, dtype=np.float32)
    W_qkv = np.asarray(W_qkv, dtype=np.float32)
    W_proj = np.asarray(W_proj, dtype=np.float32)
    b_proj = np.asarray(b_proj, dtype=np.float32)

    if "nc" not in _CACHE:
        _CACHE["nc"] = _build_program()
    nc = _CACHE["nc"]

    in_maps = _prep_inputs(x, W_qkv, W_proj)
    res = run_bass_kernel_spmd(nc, in_maps, core_ids=list(range(NCORES)))
    out = np.empty((B, T, C), dtype=np.float32)
    for b in range(B):
        out[b] = res.results[2 * b]["y"] + res.results[2 * b + 1]["y"] + b_proj
    return out


# revision 23
# speedup vs baseline: 1.1024x; 1.0885x over previous
"""Multi-head causal attention (B=4, T=2048, C=1024, H=16, HD=64) on 8 TRN2 NeuronCores.

Sharding: core c handles batch b = c//2 and heads hg*8..hg*8+8 where hg = c%2
(data parallel on B, tensor parallel on heads). Each core computes
qkv projection for its head group, causal attention for its 8 heads, and a
partial output projection over its 512 local channels. Host sums the two
partial projections per batch and adds the bias.

Per-core device layouts (all chosen so no on-chip transposes of x/W are needed):
  xT    [C=1024, T=2048] bf16   (x[b].T, host-transposed)
  wqkT  [C=1024, 1024]   bf16   (rows: Q of 8 heads then K of 8 heads, transposed;
                                 Q part pre-scaled by HD^-0.5)
  wvT   [C=1024, 512]    bf16
  wpT   [512, 1024]      bf16   (W_proj columns for local channels, transposed)
Stages:
  1. qkT[o, t] = W_sel @ x.T  (o: 512 Q + 512 K, head-pair h'=2p at partitions
     0-63 / 64-127 of chunk p)  and V [t, (h, d+ones)] natural with a ones
     column appended per head.
  2. Attention computes S^T directly: for each (pair, i-super of 512, j-tile
     of 128): S^T[j, i] = K_tile^T Q (row-tiled K=64 matmul pair), causal mask
     add on the diagonal block, exp on ACT (no accumulation needed) giving
     P^T[j, i] in SBUF bf16. AV accumulates O[i, d] (+ row-sum denominator via
     the ones column) with cheap N=65 matmuls: O_psum[i-tile] += P^T_tile^T
     @ [V|1]. After the j-loop, rows are normalized by the reciprocal of the
     denominator on DVE and O is DMA-transposed ([128 t, 2x64 hd] blocks) into
     oT[d, t] for the projection.
  3. y[t, :] = O @ WpT (bf16) accumulated in fp32 PSUM.
"""

import numpy as np
import ml_dtypes

B, T, C = 4, 2048, 1024
H = 16
HD = 64
NCORES = 8
P = 128

_CACHE = {}


def _build_program():
    import concourse.bass as bass
    import concourse.mybir as mybir
    import concourse.tile as tile
    from concourse import bacc
    from contextlib import ExitStack

    DT_BF = mybir.dt.bfloat16
    DT_F32 = mybir.dt.float32
    Exp = mybir.ActivationFunctionType.Exp

    nc = bacc.Bacc("TRN2", target_bir_lowering=False, num_devices=NCORES)
    xT = nc.dram_tensor("xT", [C, T], DT_BF, kind="ExternalInput")
    wqkT = nc.dram_tensor("wqkT", [C, 1024], DT_BF, kind="ExternalInput")
    wvT = nc.dram_tensor("wvT", [C, 512], DT_BF, kind="ExternalInput")
    wpT = nc.dram_tensor("wpT", [512, 1024], DT_BF, kind="ExternalInput")
    y = nc.dram_tensor("y", [T, C], DT_F32, kind="ExternalOutput")

    NT = T // P            # 16 t-tiles
    NSUP = 4               # i-supers of 512
    NPAIR = 4              # head pairs per core
    LAG = 2                # AV trails exp by this many j-tiles

    with tile.TileContext(nc) as tc, ExitStack() as ctx:
        pers = ctx.enter_context(tc.tile_pool(name="pers", bufs=1))
        xtp = ctx.enter_context(tc.tile_pool(name="xtp", bufs=2))
        ptp = ctx.enter_context(tc.tile_pool(name="ptp", bufs=2))
        obp = ctx.enter_context(tc.tile_pool(name="obp", bufs=2))
        worky = ctx.enter_context(tc.tile_pool(name="worky", bufs=4))
        small = ctx.enter_context(tc.tile_pool(name="small", bufs=8))
        sps = ctx.enter_context(tc.tile_pool(name="sps", bufs=2, space="PSUM"))
        avp = ctx.enter_context(tc.tile_pool(name="av", bufs=2, space="PSUM"))
        accp = ctx.enter_context(tc.tile_pool(name="acc", bufs=2, space="PSUM"))

        # ---- loads ----
        xt0 = xtp.tile([P, 4, T], DT_BF, tag="xtp")
        xt1 = xtp.tile([P, 4, T], DT_BF, tag="xtp")
        xtre = xT.rearrange("(o p) t -> p o t", p=P)

        def xt(cc):
            return (xt0 if cc < 4 else xt1)[:, cc % 4, :]
        wqk = pers.tile([P, 8, 1024], DT_BF, tag="wqk")
        wv = pers.tile([P, 8, 512], DT_BF, tag="wv")
        wqkre = wqkT.rearrange("(o p) f -> p o f", p=P)
        wvre = wvT.rearrange("(o p) f -> p o f", p=P)
        TH = T // 2
        for i in range(8):
            nc.sync.dma_start(wqk[:, i, :], wqkre[:, i, :])
            nc.sync.dma_start(xt(i)[:, :TH], xtre[:, i, :TH])
        for i in range(8):
            nc.sync.dma_start(wv[:, i, :], wvre[:, i, :])
            nc.sync.dma_start(xt(i)[:, TH:], xtre[:, i, TH:])
        wp = pers.tile([P, 4, 1024], DT_BF, tag="wp")

        # maskT2[j, hh, i] = -1e30 where j > i (S^T orientation: partition=j,
        # free=i), replicated for both heads so one DVE add masks both.
        maskT2 = pers.tile([P, 2, P], DT_F32, tag="maskT2")
        nc.gpsimd.memset(maskT2[:], 0.0)
        for hh in range(2):
            nc.gpsimd.affine_select(
                out=maskT2[:, hh, :],
                in_=maskT2[:, hh, :],
                compare_op=mybir.AluOpType.is_ge,
                fill=-1e30,
                base=0,
                # keep (i - j) >= 0, i.e. j <= i
                pattern=[[1, P]],
                channel_multiplier=-1,
            )

        # ---- stage 1: qkT [o, t] and V [t, (h, d|1)] per t-chunk ----
        qkT = pers.tile([P, 8, T], DT_BF, tag="qkT")
        v2 = pers.tile([P, NT, 8, 65], DT_BF, tag="v2")
        nc.gpsimd.memset(v2[:], 1.0)

        def qkv_groups(tc4):
            ts = slice(tc4 * 512, (tc4 + 1) * 512)

            def qk_group(oc):
                def go():
                    acc = accp.tile([P, 512], DT_F32, tag="acc")
                    for cc in range(8):
                        nc.tensor.matmul(
                            acc[:],
                            wqk[:, cc, oc * P:(oc + 1) * P],
                            xt(cc)[:, ts],
                            start=(cc == 0), stop=(cc == 7),
                        )
                    nc.vector.tensor_copy(qkT[:, oc, ts], acc[:])
                return go

            def v_group(tt):
                def go():
                    accv = accp.tile([P, 512], DT_F32, tag="acc")
                    for cc in range(8):
                        nc.tensor.matmul(
                            accv[:],
                            xt(cc)[:, tt * P:(tt + 1) * P],
                            wv[:, cc, :],
                            start=(cc == 0), stop=(cc == 7),
                        )
                    nc.vector.tensor_copy(
                        v2[:, tt, :, 0:64],
                        accv[:].rearrange("p (h d) -> p h d", d=64))
                return go

            return ([qk_group(oc) for oc in range(4, 8)]
                    + [qk_group(oc) for oc in range(0, 4)]
                    + [v_group(tt) for tt in range(tc4 * 4, tc4 * 4 + 4)])

        # oT keeps only 2 i-supers (written by chains of sup, read by the
        # projection woven into sup+1): [p, sup%2, hc, t_local]
        oT = pers.tile([P, 2, 4, 512], DT_BF, tag="oT")

        def chain(sup, pr, fillers):
            """Attention j-loop for head pair pr, query rows [sup*512, ...).
            Returns a `finish` closure (AV burst + normalize + transpose) that
            the caller weaves into the NEXT chain's fillers, so the PE runs the
            next pair's QK while ACT drains this pair's last exps."""
            qp = qkT[:, pr, :]        # [128, T]: head A at part 0-63, B at 64-127
            kp = qkT[:, 4 + pr, :]
            i0 = sup * 512
            njt = 4 * sup + 4
            oH = [avp.tile([P, 512], DT_F32, tag="av", name=f"oH{sup}_{pr}_{h}")
                  for h in range(2)]
            pTall = ptp.tile([P, njt, 2, 512], DT_BF, tag="ptp",
                             name=f"pTall{sup}_{pr}")
            nfil = len(fillers)
            fi = 0

            for jt in range(njt):
                # paced filler (before the gated QK so PE has queued work)
                while fi * njt < (jt + 1) * nfil:
                    fillers[fi]()
                    fi += 1
                ext_start = max(i0, jt * P)
                ext = i0 + 512 - ext_start
                sp = sps.tile([P, 2, 512], DT_F32, tag="sps", name=f"sp{jt}")
                for hh in range(2):
                    hsl = slice(hh * 64, hh * 64 + 64)
                    nc.tensor.matmul(
                        sp[:, hh, :ext],
                        kp[hsl, jt * P:(jt + 1) * P],
                        qp[hsl, ext_start:ext_start + ext],
                        start=True, stop=True,
                    )
                if jt >= 4 * sup:  # diagonal block at chunk cols 0:128
                    nc.vector.tensor_add(sp[:, :, 0:P], sp[:, :, 0:P], maskT2[:])
                nc.scalar.activation(pTall[:, jt, :, :ext], sp[:, :, :ext], Exp)
            while fi < nfil:
                fillers[fi]()
                fi += 1

            def finish():
                # AV: per (i-tile, head) a contiguous accumulation group over
                # j. Groups sharing a PSUM bank must not interleave
                # (start=True marks the whole 2KB zero region), so bursts run
                # group-by-group.
                for itl in range(4):
                    itg = sup * 4 + itl
                    for hh in range(2):
                        for jt in range(itg + 1):
                            ext_start = max(i0, jt * P)
                            off = i0 + itl * P - ext_start
                            nc.tensor.matmul(
                                oH[hh][:, itl * P:itl * P + 65],
                                pTall[:, jt, hh, off:off + P],
                                v2[:, jt, 2 * pr + hh, :],
                                start=(jt == 0), stop=(jt == itg),
                            )
                # normalize rows by the ones-column denominator, pack, and
                # transpose into oT for the projection
                rc = small.tile([P, 8], DT_F32, tag="rc")
                ob = obp.tile([P, 8, 64], DT_BF, tag="ob")  # [t, (itl, h), d]
                for hh in range(2):
                    den = oH[hh][:].rearrange("p (i c) -> p i c", c=P)[:, :, 64]
                    nc.vector.reciprocal_approx_fast(rc[:, hh * 4:hh * 4 + 4], den)
                    for itl in range(4):
                        nc.vector.tensor_scalar_mul(
                            ob[:, itl * 2 + hh, :],
                            oH[hh][:, itl * P:itl * P + 64],
                            rc[:, hh * 4 + itl:hh * 4 + itl + 1])
                for itl in range(4):
                    nc.sync.dma_start_transpose(
                        oT[:, sup % 2, pr, itl * P:(itl + 1) * P],
                        ob[:, itl * 2:itl * 2 + 2, :])
            return finish

        def proj_groups(sup):
            def group(tt, oc2):
                def go():
                    ysb = worky.tile([P, 512], DT_F32, tag="ysb")
                    acc = accp.tile([P, 512], DT_F32, tag="acc")
                    for hc in range(4):
                        nc.tensor.matmul(
                            acc[:],
                            oT[:, sup % 2, hc, (tt - sup * 4) * P:
                               (tt - sup * 4 + 1) * P],
                            wp[:, hc, oc2 * 512:(oc2 + 1) * 512],
                            start=(hc == 0), stop=(hc == 3),
                        )
                    nc.vector.tensor_copy(ysb[:], acc[:])
                    nc.gpsimd.dma_start(
                        y[tt * P:(tt + 1) * P, oc2 * 512:(oc2 + 1) * 512], ysb[:])
                return go
            return [group(tt, oc2)
                    for tt in range(sup * 4, sup * 4 + 4) for oc2 in range(2)]

        wpre = wpT.rearrange("(o p) f -> p o f", p=P)
        pending = None
        for sup in range(NSUP):
            if sup == 0:
                g0 = qkv_groups(0)  # [k4..7, q0..3, v0..3]
                k_, q_, v_ = g0[0:4], g0[4:8], g0[8:12]
                pre = [[k_[0], q_[0]],
                       [k_[1], q_[1]],
                       [k_[2], q_[2]],
                       [k_[3], q_[3]]]
                # v0..3 must be chain(0,0)'s fillers: its AV burst reads all
                # four v2 chunks, and fillers flush before the burst.
                filler = v_ + qkv_groups(1)
            else:
                pre = [[], [], [], []]
                filler = qkv_groups(sup + 1) if sup < NSUP - 1 else []
                filler = filler + proj_groups(sup - 1)
            nfil = (len(filler) + NPAIR - 1) // NPAIR if filler else 0
            for pr in range(NPAIR):
                for g in pre[pr]:
                    g()
                sl = filler[pr * nfil:(pr + 1) * nfil]
                if pending is not None:
                    # weave the previous chain's finish into this chain's
                    # fillers; at (sup3, pr0) it must precede the proj(2)
                    # units whose oT inputs it writes
                    idx = min(1 if (sup < 3 or pr > 0) else 0, len(sl))
                    sl = sl[:idx] + [pending] + sl[idx:]
                pending = chain(sup, pr, sl)
            if sup == 0:
                for i in range(4):
                    nc.sync.dma_start(wp[:, i, :], wpre[:, i, :])
        pending()
        for g in proj_groups(NSUP - 1):
            g()

    nc.compile()
    return nc


def _prep_inputs(x, W_qkv, W_proj):
    """Per-core host-side sharding and layout prep."""
    bf16 = ml_dtypes.bfloat16
    scale = np.float32(HD ** -0.5)
    in_maps = []
    for c in range(NCORES):
        b, hg = c // 2, c % 2
        heads = list(range(hg * 8, hg * 8 + 8))
        rq = np.concatenate([np.arange(h * 192, h * 192 + 64) for h in heads])
        rk = np.concatenate([np.arange(h * 192 + 64, h * 192 + 128) for h in heads])
        rv = np.concatenate([np.arange(h * 192 + 128, h * 192 + 192) for h in heads])
        wq = W_qkv[rq] * scale           # fold softmax scale into Q (exact: /8)
        wk = W_qkv[rk]
        wqkT = np.ascontiguousarray(np.concatenate([wq, wk], 0).T).astype(bf16)
        wvT = np.ascontiguousarray(W_qkv[rv].T).astype(bf16)
        wpT = np.ascontiguousarray(W_proj[:, hg * 512:(hg + 1) * 512].T)
        xTb = np.ascontiguousarray(x[b].T).astype(bf16)
        in_maps.append({"xT": xTb, "wqkT": wqkT, "wvT": wvT,
                        "wpT": wpT.astype(bf16)})
    return in_maps


def kernel(x, W_qkv, W_proj, b_proj):
    from concourse.bass_utils import run_bass_kernel_spmd

    x = np.asarray(x, dtype=np.float32)
    W_qkv = np.asarray(W_qkv, dtype=np.float32)
    W_proj = np.asarray(W_proj, dtype=np.float32)
    b_proj = np.asarray(b_proj, dtype=np.float32)

    if "nc" not in _CACHE:
        _CACHE["nc"] = _build_program()
    nc = _CACHE["nc"]

    in_maps = _prep_inputs(x, W_qkv, W_proj)
    res = run_bass_kernel_spmd(nc, in_maps, core_ids=list(range(NCORES)))
    out = np.empty((B, T, C), dtype=np.float32)
    for b in range(B):
        out[b] = res.results[2 * b]["y"] + res.results[2 * b + 1]["y"] + b_proj
    return out


# revision 25
# speedup vs baseline: 1.1130x; 1.0095x over previous
"""Multi-head causal attention (B=4, T=2048, C=1024, H=16, HD=64) on 8 TRN2 NeuronCores.

Sharding: core c handles batch b = c//2 and heads hg*8..hg*8+8 where hg = c%2
(data parallel on B, tensor parallel on heads). Each core computes
qkv projection for its head group, causal attention for its 8 heads, and a
partial output projection over its 512 local channels. Host sums the two
partial projections per batch and adds the bias.

Per-core device layouts (all chosen so no on-chip transposes of x/W are needed):
  xT    [C=1024, T=2048] bf16   (x[b].T, host-transposed)
  wqkT  [C=1024, 1024]   bf16   (rows: Q of 8 heads then K of 8 heads, transposed;
                                 Q part pre-scaled by HD^-0.5)
  wvT   [C=1024, 512]    bf16
  wpT   [512, 1024]      bf16   (W_proj columns for local channels, transposed)
Stages:
  1. qkT[o, t] = W_sel @ x.T  (o: 512 Q + 512 K, head-pair h'=2p at partitions
     0-63 / 64-127 of chunk p)  and V [t, (h, d+ones)] natural with a ones
     column appended per head.
  2. Attention computes S^T directly: for each (pair, i-super of 512, j-tile
     of 128): S^T[j, i] = K_tile^T Q (row-tiled K=64 matmul pair), causal mask
     add on the diagonal block, exp on ACT (no accumulation needed) giving
     P^T[j, i] in SBUF bf16. AV accumulates O[i, d] (+ row-sum denominator via
     the ones column) with cheap N=65 matmuls: O_psum[i-tile] += P^T_tile^T
     @ [V|1]. After the j-loop, rows are normalized by the reciprocal of the
     denominator on DVE and O is DMA-transposed ([128 t, 2x64 hd] blocks) into
     oT[d, t] for the projection.
  3. y[t, :] = O @ WpT (bf16) accumulated in fp32 PSUM.
"""

import numpy as np
import ml_dtypes

B, T, C = 4, 2048, 1024
H = 16
HD = 64
NCORES = 8
P = 128

_CACHE = {}


def _build_program():
    import concourse.bass as bass
    import concourse.mybir as mybir
    import concourse.tile as tile
    from concourse import bacc
    from contextlib import ExitStack

    DT_BF = mybir.dt.bfloat16
    DT_F32 = mybir.dt.float32
    Exp = mybir.ActivationFunctionType.Exp

    nc = bacc.Bacc("TRN2", target_bir_lowering=False, num_devices=NCORES)
    xT = nc.dram_tensor("xT", [C, T], DT_BF, kind="ExternalInput")
    wqkT = nc.dram_tensor("wqkT", [C, 1024], DT_BF, kind="ExternalInput")
    wvT = nc.dram_tensor("wvT", [C, 512], DT_BF, kind="ExternalInput")
    wpT = nc.dram_tensor("wpT", [512, 1024], DT_BF, kind="ExternalInput")
    y = nc.dram_tensor("y", [T, C], DT_F32, kind="ExternalOutput")

    NT = T // P            # 16 t-tiles
    NSUP = 4               # i-supers of 512
    NPAIR = 4              # head pairs per core
    LAG = 2                # AV trails exp by this many j-tiles

    with tile.TileContext(nc) as tc, ExitStack() as ctx:
        pers = ctx.enter_context(tc.tile_pool(name="pers", bufs=1))
        xtp = ctx.enter_context(tc.tile_pool(name="xtp", bufs=2))
        ptp = ctx.enter_context(tc.tile_pool(name="ptp", bufs=2))
        obp = ctx.enter_context(tc.tile_pool(name="obp", bufs=2))
        worky = ctx.enter_context(tc.tile_pool(name="worky", bufs=4))
        small = ctx.enter_context(tc.tile_pool(name="small", bufs=8))
        sps = ctx.enter_context(tc.tile_pool(name="sps", bufs=2, space="PSUM"))
        avp = ctx.enter_context(tc.tile_pool(name="av", bufs=2, space="PSUM"))
        accp = ctx.enter_context(tc.tile_pool(name="acc", bufs=2, space="PSUM"))

        # ---- loads ----
        xt0 = xtp.tile([P, 4, T], DT_BF, tag="xtp")
        xt1 = xtp.tile([P, 4, T], DT_BF, tag="xtp")
        xtre = xT.rearrange("(o p) t -> p o t", p=P)

        def xt(cc):
            return (xt0 if cc < 4 else xt1)[:, cc % 4, :]
        wqk = pers.tile([P, 8, 1024], DT_BF, tag="wqk")
        wv = pers.tile([P, 8, 512], DT_BF, tag="wv")
        wqkre = wqkT.rearrange("(o p) f -> p o f", p=P)
        wvre = wvT.rearrange("(o p) f -> p o f", p=P)
        TH = T // 2
        for i in range(8):
            nc.sync.dma_start(wqk[:, i, :], wqkre[:, i, :])
            nc.sync.dma_start(xt(i)[:, :TH], xtre[:, i, :TH])
        for i in range(8):
            nc.sync.dma_start(wv[:, i, :], wvre[:, i, :])
            nc.sync.dma_start(xt(i)[:, TH:], xtre[:, i, TH:])
        wp = pers.tile([P, 4, 1024], DT_BF, tag="wp")

        # maskT2[j, hh, i] = -1e30 where j > i (S^T orientation: partition=j,
        # free=i), replicated for both heads so one DVE add masks both.
        maskT2 = pers.tile([P, 2, P], DT_F32, tag="maskT2")
        nc.gpsimd.memset(maskT2[:], 0.0)
        for hh in range(2):
            nc.gpsimd.affine_select(
                out=maskT2[:, hh, :],
                in_=maskT2[:, hh, :],
                compare_op=mybir.AluOpType.is_ge,
                fill=-1e30,
                base=0,
                # keep (i - j) >= 0, i.e. j <= i
                pattern=[[1, P]],
                channel_multiplier=-1,
            )

        # ---- stage 1: qkT [o, t] and V [t, (h, d|1)] per t-chunk ----
        qkT = pers.tile([P, 8, T], DT_BF, tag="qkT")
        v2 = pers.tile([P, NT, 8, 65], DT_BF, tag="v2")
        nc.gpsimd.memset(v2[:], 1.0)

        def qkv_groups(tc4):
            ts = slice(tc4 * 512, (tc4 + 1) * 512)

            def qk_group(oc):
                def go():
                    acc = accp.tile([P, 512], DT_F32, tag="acc")
                    for cc in range(8):
                        nc.tensor.matmul(
                            acc[:],
                            wqk[:, cc, oc * P:(oc + 1) * P],
                            xt(cc)[:, ts],
                            start=(cc == 0), stop=(cc == 7),
                        )
                    nc.vector.tensor_copy(qkT[:, oc, ts], acc[:])
                return go

            def v_group(tt):
                def go():
                    accv = accp.tile([P, 512], DT_F32, tag="acc")
                    for cc in range(8):
                        nc.tensor.matmul(
                            accv[:],
                            xt(cc)[:, tt * P:(tt + 1) * P],
                            wv[:, cc, :],
                            start=(cc == 0), stop=(cc == 7),
                        )
                    nc.vector.tensor_copy(
                        v2[:, tt, :, 0:64],
                        accv[:].rearrange("p (h d) -> p h d", d=64))
                return go

            return ([qk_group(oc) for oc in range(4, 8)]
                    + [qk_group(oc) for oc in range(0, 4)]
                    + [v_group(tt) for tt in range(tc4 * 4, tc4 * 4 + 4)])

        # oT keeps only 2 i-supers (written by chains of sup, read by the
        # projection woven into sup+1): [p, sup%2, hc, t_local]
        oT = pers.tile([P, 2, 4, 512], DT_BF, tag="oT")

        def chain(sup, pr, fillers):
            """Attention j-loop for head pair pr, query rows [sup*512, ...).
            Returns a `finish` closure (AV burst + normalize + transpose) that
            the caller weaves into the NEXT chain's fillers, so the PE runs the
            next pair's QK while ACT drains this pair's last exps."""
            qp = qkT[:, pr, :]        # [128, T]: head A at part 0-63, B at 64-127
            kp = qkT[:, 4 + pr, :]
            i0 = sup * 512
            njt = 4 * sup + 4
            oH = [avp.tile([P, 512], DT_F32, tag="av", name=f"oH{sup}_{pr}_{h}")
                  for h in range(2)]
            pTall = ptp.tile([P, njt, 2, 512], DT_BF, tag="ptp",
                             name=f"pTall{sup}_{pr}")
            nfil = len(fillers)
            fi = 0

            for jt in range(njt):
                # paced filler; starts at jt=1 so this chain's first QK is
                # queued on the PE before any woven-in previous finish
                while fi * njt < jt * nfil:
                    fillers[fi]()
                    fi += 1
                ext_start = max(i0, jt * P)
                ext = i0 + 512 - ext_start
                sp = sps.tile([P, 2, 512], DT_F32, tag="sps", name=f"sp{jt}")
                for hh in range(2):
                    hsl = slice(hh * 64, hh * 64 + 64)
                    nc.tensor.matmul(
                        sp[:, hh, :ext],
                        kp[hsl, jt * P:(jt + 1) * P],
                        qp[hsl, ext_start:ext_start + ext],
                        start=True, stop=True,
                    )
                if jt >= 4 * sup:  # diagonal block at chunk cols 0:128
                    nc.vector.tensor_add(sp[:, :, 0:P], sp[:, :, 0:P], maskT2[:])
                nc.scalar.activation(pTall[:, jt, :, :ext], sp[:, :, :ext], Exp)
            while fi < nfil:
                fillers[fi]()
                fi += 1

            def finish():
                # AV: per (i-tile, head) a contiguous accumulation group over
                # j. Groups sharing a PSUM bank must not interleave
                # (start=True marks the whole 2KB zero region), so bursts run
                # group-by-group. Normalization (by the ones-column
                # denominator) and the oT transpose go out per i-tile so the
                # last transpose lands right after the last group.
                rc = small.tile([P, 8], DT_F32, tag="rc")
                ob = obp.tile([P, 8, 64], DT_BF, tag="ob")  # [t, (itl, h), d]
                for itl in range(4):
                    itg = sup * 4 + itl
                    for hh in range(2):
                        for jt in range(itg + 1):
                            ext_start = max(i0, jt * P)
                            off = i0 + itl * P - ext_start
                            nc.tensor.matmul(
                                oH[hh][:, itl * P:itl * P + 65],
                                pTall[:, jt, hh, off:off + P],
                                v2[:, jt, 2 * pr + hh, :],
                                start=(jt == 0), stop=(jt == itg),
                            )
                    for hh in range(2):
                        k = itl * 2 + hh
                        nc.vector.reciprocal_approx_fast(
                            rc[:, k:k + 1],
                            oH[hh][:, itl * P + 64:itl * P + 65])
                        nc.vector.tensor_scalar_mul(
                            ob[:, k, :],
                            oH[hh][:, itl * P:itl * P + 64],
                            rc[:, k:k + 1])
                    nc.sync.dma_start_transpose(
                        oT[:, sup % 2, pr, itl * P:(itl + 1) * P],
                        ob[:, itl * 2:itl * 2 + 2, :])
            return finish

        def proj_groups(sup):
            def group(tt, oc2):
                def go():
                    ysb = worky.tile([P, 512], DT_F32, tag="ysb")
                    acc = accp.tile([P, 512], DT_F32, tag="acc")
                    for hc in range(4):
                        nc.tensor.matmul(
                            acc[:],
                            oT[:, sup % 2, hc, (tt - sup * 4) * P:
                               (tt - sup * 4 + 1) * P],
                            wp[:, hc, oc2 * 512:(oc2 + 1) * 512],
                            start=(hc == 0), stop=(hc == 3),
                        )
                    nc.vector.tensor_copy(ysb[:], acc[:])
                    nc.gpsimd.dma_start(
                        y[tt * P:(tt + 1) * P, oc2 * 512:(oc2 + 1) * 512], ysb[:])
                return go
            return [group(tt, oc2)
                    for tt in range(sup * 4, sup * 4 + 4) for oc2 in range(2)]

        wpre = wpT.rearrange("(o p) f -> p o f", p=P)
        pending = None
        for sup in range(NSUP):
            if sup == 0:
                g0 = qkv_groups(0)  # [k4..7, q0..3, v0..3]
                k_, q_, v_ = g0[0:4], g0[4:8], g0[8:12]
                pre = [[k_[0], q_[0]],
                       [k_[1], q_[1]],
                       [k_[2], q_[2]],
                       [k_[3], q_[3]]]
                # v0..3 must be chain(0,0)'s fillers: its AV burst reads all
                # four v2 chunks, and fillers flush before the burst.
                filler = v_ + qkv_groups(1)
            else:
                pre = [[], [], [], []]
                filler = qkv_groups(sup + 1) if sup < NSUP - 1 else []
                filler = filler + proj_groups(sup - 1)
            nfil = (len(filler) + NPAIR - 1) // NPAIR if filler else 0
            for pr in range(NPAIR):
                for g in pre[pr]:
                    g()
                sl = filler[pr * nfil:(pr + 1) * nfil]
                if pending is not None:
                    # weave the previous chain's finish into this chain's
                    # fillers; at (sup3, pr0) it must precede the proj(2)
                    # units whose oT inputs it writes
                    idx = min(1 if (sup < 3 or pr > 0) else 0, len(sl))
                    sl = sl[:idx] + [pending] + sl[idx:]
                pending = chain(sup, pr, sl)
            if sup == 0:
                for i in range(4):
                    nc.sync.dma_start(wp[:, i, :], wpre[:, i, :])
        pending()
        for g in proj_groups(NSUP - 1):
            g()

    nc.compile()
    return nc


def _prep_inputs(x, W_qkv, W_proj):
    """Per-core host-side sharding and layout prep."""
    bf16 = ml_dtypes.bfloat16
    scale = np.float32(HD ** -0.5)
    in_maps = []
    for c in range(NCORES):
        b, hg = c // 2, c % 2
        heads = list(range(hg * 8, hg * 8 + 8))
        rq = np.concatenate([np.arange(h * 192, h * 192 + 64) for h in heads])
        rk = np.concatenate([np.arange(h * 192 + 64, h * 192 + 128) for h in heads])
        rv = np.concatenate([np.arange(h * 192 + 128, h * 192 + 192) for h in heads])
        wq = W_qkv[rq] * scale           # fold softmax scale into Q (exact: /8)
        wk = W_qkv[rk]
        wqkT = np.ascontiguousarray(np.concatenate([wq, wk], 0).T).astype(bf16)
        wvT = np.ascontiguousarray(W_qkv[rv].T).astype(bf16)
        wpT = np.ascontiguousarray(W_proj[:, hg * 512:(hg + 1) * 512].T)
        xTb = np.ascontiguousarray(x[b].T).astype(bf16)
        in_maps.append({"xT": xTb, "wqkT": wqkT, "wvT": wvT,
                        "wpT": wpT.astype(bf16)})
    return in_maps


def kernel(x, W_qkv, W_proj, b_proj):
    from concourse.bass_utils import run_bass_kernel_spmd

    x = np.asarray(x, dtype=np.float32)
    W_qkv = np.asarray(W_qkv, dtype=np.float32)
    W_proj = np.asarray(W_proj, dtype=np.float32)
    b_proj = np.asarray(b_proj, dtype=np.float32)

    if "nc" not in _CACHE:
        _CACHE["nc"] = _build_program()
    nc = _CACHE["nc"]

    in_maps = _prep_inputs(x, W_qkv, W_proj)
    res = run_bass_kernel_spmd(nc, in_maps, core_ids=list(range(NCORES)))
    out = np.empty((B, T, C), dtype=np.float32)
    for b in range(B):
        out[b] = res.results[2 * b]["y"] + res.results[2 * b + 1]["y"] + b_proj
    return out


# revision 26
# speedup vs baseline: 1.1288x; 1.0142x over previous
"""Multi-head causal attention (B=4, T=2048, C=1024, H=16, HD=64) on 8 TRN2 NeuronCores.

Sharding: core c handles batch b = c//2 and heads hg*8..hg*8+8 where hg = c%2
(data parallel on B, tensor parallel on heads). Each core computes
qkv projection for its head group, causal attention for its 8 heads, and a
partial output projection over its 512 local channels. Host sums the two
partial projections per batch and adds the bias.

Per-core device layouts (all chosen so no on-chip transposes of x/W are needed):
  xT    [C=1024, T=2048] bf16   (x[b].T, host-transposed)
  wqkT  [C=1024, 1024]   bf16   (rows: Q of 8 heads then K of 8 heads, transposed;
                                 Q part pre-scaled by HD^-0.5)
  wvT   [C=1024, 512]    bf16
  wpT   [512, 1024]      bf16   (W_proj columns for local channels, transposed)
Stages:
  1. qkT[o, t] = W_sel @ x.T  (o: 512 Q + 512 K, head-pair h'=2p at partitions
     0-63 / 64-127 of chunk p)  and V [t, (h, d+ones)] natural with a ones
     column appended per head.
  2. Attention computes S^T directly: for each (pair, i-super of 512, j-tile
     of 128): S^T[j, i] = K_tile^T Q (row-tiled K=64 matmul pair), causal mask
     add on the diagonal block, exp on ACT (no accumulation needed) giving
     P^T[j, i] in SBUF bf16. AV accumulates O[i, d] (+ row-sum denominator via
     the ones column) with cheap N=65 matmuls: O_psum[i-tile] += P^T_tile^T
     @ [V|1]. After the j-loop, rows are normalized by the reciprocal of the
     denominator on DVE and O is DMA-transposed ([128 t, 2x64 hd] blocks) into
     oT[d, t] for the projection.
  3. y[t, :] = O @ WpT (bf16) accumulated in fp32 PSUM.
"""

import numpy as np
import ml_dtypes

B, T, C = 4, 2048, 1024
H = 16
HD = 64
NCORES = 8
P = 128

_CACHE = {}


def _build_program():
    import concourse.bass as bass
    import concourse.mybir as mybir
    import concourse.tile as tile
    from concourse import bacc
    from contextlib import ExitStack

    DT_BF = mybir.dt.bfloat16
    DT_F32 = mybir.dt.float32
    Exp = mybir.ActivationFunctionType.Exp

    nc = bacc.Bacc("TRN2", target_bir_lowering=False, num_devices=NCORES)
    xT = nc.dram_tensor("xT", [C, T], DT_BF, kind="ExternalInput")
    wqkT = nc.dram_tensor("wqkT", [C, 1024], DT_BF, kind="ExternalInput")
    wvT = nc.dram_tensor("wvT", [C, 512], DT_BF, kind="ExternalInput")
    wpT = nc.dram_tensor("wpT", [512, 1024], DT_BF, kind="ExternalInput")
    y = nc.dram_tensor("y", [T, C], DT_F32, kind="ExternalOutput")

    NT = T // P            # 16 t-tiles
    NSUP = 4               # i-supers of 512
    NPAIR = 4              # head pairs per core
    LAG = 2                # AV trails exp by this many j-tiles

    with tile.TileContext(nc) as tc, ExitStack() as ctx:
        pers = ctx.enter_context(tc.tile_pool(name="pers", bufs=1))
        xtp = ctx.enter_context(tc.tile_pool(name="xtp", bufs=2))
        ptp = ctx.enter_context(tc.tile_pool(name="ptp", bufs=2))
        obp = ctx.enter_context(tc.tile_pool(name="obp", bufs=2))
        worky = ctx.enter_context(tc.tile_pool(name="worky", bufs=4))
        small = ctx.enter_context(tc.tile_pool(name="small", bufs=8))
        sps = ctx.enter_context(tc.tile_pool(name="sps", bufs=2, space="PSUM"))
        avp = ctx.enter_context(tc.tile_pool(name="av", bufs=2, space="PSUM"))
        accp = ctx.enter_context(tc.tile_pool(name="acc", bufs=2, space="PSUM"))

        # ---- loads ----
        xt0 = xtp.tile([P, 4, T], DT_BF, tag="xtp")
        xt1 = xtp.tile([P, 4, T], DT_BF, tag="xtp")
        xtre = xT.rearrange("(o p) t -> p o t", p=P)

        def xt(cc):
            return (xt0 if cc < 4 else xt1)[:, cc % 4, :]
        wqk = pers.tile([P, 8, 1024], DT_BF, tag="wqk")
        wv = pers.tile([P, 8, 512], DT_BF, tag="wv")
        wqkre = wqkT.rearrange("(o p) f -> p o f", p=P)
        wvre = wvT.rearrange("(o p) f -> p o f", p=P)
        TH = T // 2
        for i in range(8):
            nc.sync.dma_start(wqk[:, i, :], wqkre[:, i, :])
            nc.sync.dma_start(xt(i)[:, :TH], xtre[:, i, :TH])
        for i in range(8):
            nc.sync.dma_start(wv[:, i, :], wvre[:, i, :])
            nc.sync.dma_start(xt(i)[:, TH:], xtre[:, i, TH:])
        wp = pers.tile([P, 4, 1024], DT_BF, tag="wp")

        # maskT2[j, hh, i] = -1e30 where j > i (S^T orientation: partition=j,
        # free=i), replicated for both heads so one DVE add masks both.
        maskT2 = pers.tile([P, 2, P], DT_F32, tag="maskT2")
        nc.gpsimd.memset(maskT2[:], 0.0)
        for hh in range(2):
            nc.gpsimd.affine_select(
                out=maskT2[:, hh, :],
                in_=maskT2[:, hh, :],
                compare_op=mybir.AluOpType.is_ge,
                fill=-1e30,
                base=0,
                # keep (i - j) >= 0, i.e. j <= i
                pattern=[[1, P]],
                channel_multiplier=-1,
            )

        # ---- stage 1: qkT [o, t] and V [t, (h, d|1)] per t-chunk ----
        qkT = pers.tile([P, 8, T], DT_BF, tag="qkT")
        v2 = pers.tile([P, NT, 8, 65], DT_BF, tag="v2")
        nc.gpsimd.memset(v2[:], 1.0)

        def qkv_groups(tc4):
            ts = slice(tc4 * 512, (tc4 + 1) * 512)

            def qk_group(oc):
                def go():
                    acc = accp.tile([P, 512], DT_F32, tag="acc")
                    for cc in range(8):
                        nc.tensor.matmul(
                            acc[:],
                            wqk[:, cc, oc * P:(oc + 1) * P],
                            xt(cc)[:, ts],
                            start=(cc == 0), stop=(cc == 7),
                        )
                    nc.vector.tensor_copy(qkT[:, oc, ts], acc[:])
                return go

            def v_group(tt):
                def go():
                    accv = accp.tile([P, 512], DT_F32, tag="acc")
                    for cc in range(8):
                        nc.tensor.matmul(
                            accv[:],
                            xt(cc)[:, tt * P:(tt + 1) * P],
                            wv[:, cc, :],
                            start=(cc == 0), stop=(cc == 7),
                        )
                    nc.vector.tensor_copy(
                        v2[:, tt, :, 0:64],
                        accv[:].rearrange("p (h d) -> p h d", d=64))
                return go

            return ([qk_group(oc) for oc in range(4, 8)]
                    + [qk_group(oc) for oc in range(0, 4)]
                    + [v_group(tt) for tt in range(tc4 * 4, tc4 * 4 + 4)])

        # oT keeps 3 i-supers (written by chains of sup, read by the
        # projection woven in 1-2 sups later): [p, sup%3, hc, t_local]
        oT = pers.tile([P, 3, 4, 512], DT_BF, tag="oT")

        def chain(sup, pr, fillers):
            """Attention j-loop for head pair pr, query rows [sup*512, ...).
            Returns a `finish` closure (AV burst + normalize + transpose) that
            the caller weaves into the NEXT chain's fillers, so the PE runs the
            next pair's QK while ACT drains this pair's last exps."""
            qp = qkT[:, pr, :]        # [128, T]: head A at part 0-63, B at 64-127
            kp = qkT[:, 4 + pr, :]
            i0 = sup * 512
            njt = 4 * sup + 4
            oH = [avp.tile([P, 512], DT_F32, tag="av", name=f"oH{sup}_{pr}_{h}")
                  for h in range(2)]
            pTall = ptp.tile([P, njt, 2, 512], DT_BF, tag="ptp",
                             name=f"pTall{sup}_{pr}")
            nfil = len(fillers)
            fi = 0

            for jt in range(njt):
                # paced filler; starts at jt=1 so this chain's first QK is
                # queued on the PE before any woven-in previous finish
                while fi * njt < jt * nfil:
                    fillers[fi]()
                    fi += 1
                ext_start = max(i0, jt * P)
                ext = i0 + 512 - ext_start
                sp = sps.tile([P, 2, 512], DT_F32, tag="sps", name=f"sp{jt}")
                for hh in range(2):
                    hsl = slice(hh * 64, hh * 64 + 64)
                    nc.tensor.matmul(
                        sp[:, hh, :ext],
                        kp[hsl, jt * P:(jt + 1) * P],
                        qp[hsl, ext_start:ext_start + ext],
                        start=True, stop=True,
                    )
                if jt >= 4 * sup:  # diagonal block at chunk cols 0:128
                    nc.vector.tensor_add(sp[:, :, 0:P], sp[:, :, 0:P], maskT2[:])
                nc.scalar.activation(pTall[:, jt, :, :ext], sp[:, :, :ext], Exp)
            while fi < nfil:
                fillers[fi]()
                fi += 1

            def finish():
                # AV: per (i-tile, head) a contiguous accumulation group over
                # j. Groups sharing a PSUM bank must not interleave
                # (start=True marks the whole 2KB zero region), so bursts run
                # group-by-group. Normalization (by the ones-column
                # denominator) and the oT transpose go out per i-tile so the
                # last transpose lands right after the last group.
                rc = small.tile([P, 8], DT_F32, tag="rc")
                ob = obp.tile([P, 8, 64], DT_BF, tag="ob")  # [t, (itl, h), d]
                for itl in range(4):
                    itg = sup * 4 + itl
                    for hh in range(2):
                        for jt in range(itg + 1):
                            ext_start = max(i0, jt * P)
                            off = i0 + itl * P - ext_start
                            nc.tensor.matmul(
                                oH[hh][:, itl * P:itl * P + 65],
                                pTall[:, jt, hh, off:off + P],
                                v2[:, jt, 2 * pr + hh, :],
                                start=(jt == 0), stop=(jt == itg),
                            )
                    for hh in range(2):
                        k = itl * 2 + hh
                        nc.vector.reciprocal_approx_fast(
                            rc[:, k:k + 1],
                            oH[hh][:, itl * P + 64:itl * P + 65])
                        nc.vector.tensor_scalar_mul(
                            ob[:, k, :],
                            oH[hh][:, itl * P:itl * P + 64],
                            rc[:, k:k + 1])
                    nc.sync.dma_start_transpose(
                        oT[:, sup % 3, pr, itl * P:(itl + 1) * P],
                        ob[:, itl * 2:itl * 2 + 2, :])
            return finish

        def proj_groups(sup):
            def group(tt, oc2):
                def go():
                    ysb = worky.tile([P, 512], DT_F32, tag="ysb")
                    acc = accp.tile([P, 512], DT_F32, tag="acc")
                    for hc in range(4):
                        nc.tensor.matmul(
                            acc[:],
                            oT[:, sup % 3, hc, (tt - sup * 4) * P:
                               (tt - sup * 4 + 1) * P],
                            wp[:, hc, oc2 * 512:(oc2 + 1) * 512],
                            start=(hc == 0), stop=(hc == 3),
                        )
                    nc.vector.tensor_copy(ysb[:], acc[:])
                    nc.gpsimd.dma_start(
                        y[tt * P:(tt + 1) * P, oc2 * 512:(oc2 + 1) * 512], ysb[:])
                return go
            return [group(tt, oc2)
                    for tt in range(sup * 4, sup * 4 + 4) for oc2 in range(2)]

        wpre = wpT.rearrange("(o p) f -> p o f", p=P)
        pending = None
        for sup in range(NSUP):
            if sup == 0:
                g0 = qkv_groups(0)  # [k4..7, q0..3, v0..3]
                k_, q_, v_ = g0[0:4], g0[4:8], g0[8:12]
                pre = [[k_[0], q_[0]],
                       [k_[1], q_[1]],
                       [k_[2], q_[2]],
                       [k_[3], q_[3]]]
                # v0..3 must be chain(0,0)'s fillers: its AV burst reads all
                # four v2 chunks, and fillers flush before the burst.
                filler = v_ + qkv_groups(1)
            else:
                pre = [[], [], [], []]
                # rebalance: qkv(s+1) in sup s; proj(0) in sup2; proj(1) and
                # proj(2) in the ACT-bound sup3 where the PE needs filler work
                filler = qkv_groups(sup + 1) if sup < NSUP - 1 else []
                if sup == 2:
                    filler = filler + proj_groups(0)
                elif sup == 3:
                    filler = proj_groups(1) + proj_groups(2)
            nfil = (len(filler) + NPAIR - 1) // NPAIR if filler else 0
            for pr in range(NPAIR):
                for g in pre[pr]:
                    g()
                sl = filler[pr * nfil:(pr + 1) * nfil]
                if pending is not None:
                    # weave the previous chain's finish into this chain's
                    # fillers; at (sup3, pr0) it must precede the proj(2)
                    # units whose oT inputs it writes
                    idx = min(1 if (sup < 3 or pr > 0) else 0, len(sl))
                    sl = sl[:idx] + [pending] + sl[idx:]
                pending = chain(sup, pr, sl)
            if sup == 0:
                for i in range(4):
                    nc.sync.dma_start(wp[:, i, :], wpre[:, i, :])
        pending()
        for g in proj_groups(NSUP - 1):
            g()

    nc.compile()
    return nc


def _prep_inputs(x, W_qkv, W_proj):
    """Per-core host-side sharding and layout prep."""
    bf16 = ml_dtypes.bfloat16
    scale = np.float32(HD ** -0.5)
    in_maps = []
    for c in range(NCORES):
        b, hg = c // 2, c % 2
        heads = list(range(hg * 8, hg * 8 + 8))
        rq = np.concatenate([np.arange(h * 192, h * 192 + 64) for h in heads])
        rk = np.concatenate([np.arange(h * 192 + 64, h * 192 + 128) for h in heads])
        rv = np.concatenate([np.arange(h * 192 + 128, h * 192 + 192) for h in heads])
        wq = W_qkv[rq] * scale           # fold softmax scale into Q (exact: /8)
        wk = W_qkv[rk]
        wqkT = np.ascontiguousarray(np.concatenate([wq, wk], 0).T).astype(bf16)
        wvT = np.ascontiguousarray(W_qkv[rv].T).astype(bf16)
        wpT = np.ascontiguousarray(W_proj[:, hg * 512:(hg + 1) * 512].T)
        xTb = np.ascontiguousarray(x[b].T).astype(bf16)
        in_maps.append({"xT": xTb, "wqkT": wqkT, "wvT": wvT,
                        "wpT": wpT.astype(bf16)})
    return in_maps


def kernel(x, W_qkv, W_proj, b_proj):
    from concourse.bass_utils import run_bass_kernel_spmd

    x = np.asarray(x, dtype=np.float32)
    W_qkv = np.asarray(W_qkv, dtype=np.float32)
    W_proj = np.asarray(W_proj, dtype=np.float32)
    b_proj = np.asarray(b_proj, dtype=np.float32)

    if "nc" not in _CACHE:
        _CACHE["nc"] = _build_program()
    nc = _CACHE["nc"]

    in_maps = _prep_inputs(x, W_qkv, W_proj)
    res = run_bass_kernel_spmd(nc, in_maps, core_ids=list(range(NCORES)))
    out = np.empty((B, T, C), dtype=np.float32)
    for b in range(B):
        out[b] = res.results[2 * b]["y"] + res.results[2 * b + 1]["y"] + b_proj
    return out
